# revision 26
# baseline (speedup 1.0000x reference)
"""GCN layer kernel for Trainium2, 8 NeuronCores — single-launch version.

Math (identical to reference):
    deg = bincount(row);  d = 1/sqrt(deg)
    h   = x @ W.T + b
    out = d * segment_sum(d[col] * h[col], row) + d^2 * h

Rewritten as aggregate-then-transform (linear map commutes with segment sum):
    U[r]   = sum_{edges (r,c)} d_c * x_c + d_r * x_r
    cc[r]  = sum_{edges (r,c)} d_c + d_r
    out[r] = d_r * (U[r] @ W.T) + (cc[r] * d_r) * b

One SPMD launch (destinations sharded across the 8 cores, identical program,
per-core data):
  * edges sorted by (dest superblock of 4x128, source chunk of 25088, dest);
    x rows (fp16, 256B) gathered in bulk with gpsimd.dma_gather.  Gathered
    edge i lands at SBUF partition i%128, tile i//128.  Tiles are packed per
    (superblock, chunk) — NOT aligned to 128-dest blocks — so cross-core
    slot padding is only ~6%.
  * per 128-edge tile and dest block it touches, a selection matrix
    st[e, dest] = (iota==dl)*d_c (fp16, dl = dest-local within superblock,
    0..511) is built with one fused tensor_scalar (is_equal, mult) — carrying
    the per-source d_c scaling — and one PE fp16 matmul accumulates
    slab^T @ st into that block's f32 PSUM tile U^T[feat, dest].  The four
    blocks of a superblock accumulate concurrently in separate PSUM banks.
  * self term: one matmul per block with rhs = ident * d_r (diagonal of d)
    which also clears the full PSUM tile.
  * per block: U^T is already [feat, dest], so no transpose: one 128x128
    matmul with W^T, then scale by d_r and add (cc*d_r)*b (cc precomputed on
    host along with all index/degree prep).  Output written fp16, host
    upcasts.
Slot padding uses source row 0 with dl = -1 and d_c = 0 (selection column is
all zero), so padded gathers are harmless; per-(superblock,chunk) tile counts
are the max over cores, keeping shapes static across the SPMD program.
"""

import numpy as np
import sys

sys.path.insert(0, "/opt/trn_rl_repo")

import concourse.bacc as bacc
import concourse.tile as tile
from concourse import mybir
from concourse.bass_utils import run_bass_kernel_spmd
from concourse.masks import make_identity

NCORES = 8
P = 128
CHUNK = 25088  # dma_gather idx is int16: source chunks must stay < 32768 rows
SB = 4  # dest blocks per superblock (gather + tile-packing granularity)
SLAB_BUFS = 2
F32 = mybir.dt.float32
F16 = mybir.dt.float16
I16 = mybir.dt.int16

_cache = {}
LAST = {}  # populated on each kernel() call (for profiling in test.py)


def _build(meta):
    """Gather + selection-matmul segment sum + per-block W matmul."""
    din = meta["din"]
    dout = meta["dout"]
    n_y = meta["n_y"]  # padded x rows (nchunk * CHUNK)
    nblk = meta["nblk"]
    sblocks = meta["sblocks"]  # list of lists of block ids
    sb_tiles = meta["sb_tiles"]  # per sb: total tiles
    sb_calls = meta["sb_calls"]  # per sb: list of (chunk, tile_off_in_sb, ntiles)
    tile_base = meta["tile_base"]  # per sb: global tile offset
    tile_segs = meta["tile_segs"]  # per global tile: list of (j, lo, w)
    ttot = meta["ttot"]

    nc = bacc.Bacc(
        "TRN2",
        target_bir_lowering=False,
        debug=False,
        enable_asserts=False,
        num_devices=NCORES,
    )
    x_t = nc.dram_tensor("x_t", [n_y, din], F16, kind="ExternalInput").ap()
    xs_t = nc.dram_tensor("xs_t", [nblk * P, din], F16, kind="ExternalInput").ap()
    idx_t = nc.dram_tensor("idx_t", [P, ttot * 8], I16, kind="ExternalInput").ap()
    dldc_t = nc.dram_tensor("dldc_t", [P, 2 * ttot], F16, kind="ExternalInput").ap()
    dv_t = nc.dram_tensor("dv_t", [P, nblk], F32, kind="ExternalInput").ap()
    cb_t = nc.dram_tensor("cb_t", [P, nblk], F32, kind="ExternalInput").ap()
    wt_t = nc.dram_tensor("wt_t", [din, dout], F32, kind="ExternalInput").ap()
    brep_t = nc.dram_tensor("brep_t", [P, dout], F32, kind="ExternalInput").ap()
    out_t = nc.dram_tensor("out_t", [nblk * P, dout], F16, kind="ExternalOutput").ap()

    max_sb_tiles = max(sb_tiles)

    with tile.TileContext(nc) as tc:
        with (
            tc.tile_pool(name="const", bufs=1) as cpool,
            tc.tile_pool(name="slab", bufs=SLAB_BUFS) as gpool,
            tc.tile_pool(name="sel", bufs=6) as selpool,
            tc.tile_pool(name="work", bufs=3) as wpool,
            tc.tile_pool(name="psum_u", bufs=1, space="PSUM") as upool,
            tc.tile_pool(name="psum_o", bufs=2, space="PSUM") as opool,
        ):
            ident = cpool.tile([P, P], dtype=F16)
            make_identity(nc, ident[:])
            iota_i = cpool.tile([P, SB * P], dtype=mybir.dt.int32)
            nc.gpsimd.iota(
                iota_i[:], pattern=[[1, SB * P]], base=0, channel_multiplier=0
            )
            iota_f = cpool.tile([P, SB * P], dtype=F16)
            nc.vector.tensor_copy(iota_f[:], iota_i[:])
            wt_sb = cpool.tile([din, dout], dtype=F32)
            nc.sync.dma_start(out=wt_sb[:], in_=wt_t[:, :])
            brep_sb = cpool.tile([P, dout], dtype=F32)
            nc.sync.dma_start(out=brep_sb[:], in_=brep_t[:, :])
            dv_sb = cpool.tile([P, nblk], dtype=F32)
            nc.sync.dma_start(out=dv_sb[:], in_=dv_t[:, :])
            cb_sb = cpool.tile([P, nblk], dtype=F32)
            nc.sync.dma_start(out=cb_sb[:], in_=cb_t[:, :])
            # one-shot bulk loads: gather indices, dl/dc (f16 -> f32), self rows
            idx_all = cpool.tile([P, ttot * 8], dtype=I16)
            nc.sync.dma_start(out=idx_all[:], in_=idx_t[:, :])
            dldc_stg = cpool.tile([P, 2 * ttot], dtype=F16)
            nc.sync.dma_start(out=dldc_stg[:], in_=dldc_t[:, :])
            dl_f = cpool.tile([P, ttot], dtype=F32)
            nc.vector.tensor_copy(dl_f[:], dldc_stg[:, 0:ttot])
            dc_f = cpool.tile([P, ttot], dtype=F32)
            nc.vector.tensor_copy(dc_f[:], dldc_stg[:, ttot : 2 * ttot])

            xs_v = xs_t.rearrange("(t p) f -> p t f", p=P)
            xs_all = cpool.tile([P, nblk, din], dtype=F16)
            nc.sync.dma_start(out=xs_all[:], in_=xs_v[:, :, :])
            out_v = out_t.rearrange("(t p) f -> p t f", p=P)
            for sbi, blks in enumerate(sblocks):
                nt_sb = sb_tiles[sbi]
                tb = tile_base[sbi]
                nb = len(blks)
                # last edge-tile touching each block (for stop flags)
                last_t = [None] * nb
                for t_sb in range(nt_sb):
                    for (j, lo, w) in tile_segs[tb + t_sb]:
                        last_t[j] = t_sb
                slab = gpool.tile([P, max_sb_tiles, din], dtype=F16, tag="slab")
                for (c, toff, nt) in sb_calls[sbi]:
                    ni = nt * P
                    nc.gpsimd.dma_gather(
                        out_ap=slab[:, toff : toff + nt, :],
                        in_ap=x_t[c * CHUNK : (c + 1) * CHUNK, :],
                        idxs_ap=idx_all[:, (tb + toff) * 8 : (tb + toff + nt) * 8],
                        num_idxs=ni,
                        num_idxs_reg=ni,
                        elem_size=din,
                        single_packet=False,
                    )
                # PSUM tiles hold U^T: [feat, dest_local], one bank per block
                ups = []
                for j, b in enumerate(blks):
                    u = upool.tile([P, P], dtype=F32, space="PSUM", tag=f"ups{j}")
                    ups.append(u)
                    # self term first: rhs = diag(d_r); clears the whole tile
                    dd = selpool.tile([P, P], dtype=F16, tag="dd")
                    nc.vector.tensor_scalar(
                        out=dd[:],
                        in0=ident[:],
                        scalar1=dv_sb[:, b : b + 1],
                        scalar2=None,
                        op0=mybir.AluOpType.mult,
                    )
                    nc.tensor.matmul(
                        out=u[:],
                        lhsT=xs_all[:, b, :],
                        rhs=dd[:],
                        start=True,
                        stop=(last_t[j] is None),
                    )
                for t_sb in range(nt_sb):
                    for (j, lo, w) in tile_segs[tb + t_sb]:
                        st = selpool.tile([P, P], dtype=F16, tag="st")
                        nc.vector.tensor_scalar(
                            out=st[:, 0:w],
                            in0=iota_f[:, j * P + lo : j * P + lo + w],
                            scalar1=dl_f[:, tb + t_sb : tb + t_sb + 1],
                            scalar2=dc_f[:, tb + t_sb : tb + t_sb + 1],
                            op0=mybir.AluOpType.is_equal,
                            op1=mybir.AluOpType.mult,
                        )
                        nc.tensor.matmul(
                            out=ups[j][:, lo : lo + w],
                            lhsT=slab[:, t_sb, :],
                            rhs=st[:, 0:w],
                            start=False,
                            stop=(t_sb == last_t[j]),
                        )
                osb_sb = wpool.tile([P, SB, dout], dtype=F16, tag="osb")
                for j, b in enumerate(blks):
                    # U^T -> SBUF, then out = d_r * (U @ W^T) + (cc*d_r) * b
                    usb = wpool.tile([P, P], dtype=F32, tag="usb")
                    nc.scalar.activation(
                        usb[:], ups[j][:], mybir.ActivationFunctionType.Copy
                    )
                    o2 = opool.tile([P, dout], dtype=F32, space="PSUM", tag="o2")
                    nc.tensor.matmul(
                        out=o2[:], lhsT=usb[:], rhs=wt_sb[:], start=True, stop=True
                    )
                    t1 = wpool.tile([P, dout], dtype=F32, tag="t1")
                    nc.scalar.activation(
                        t1[:],
                        brep_sb[:],
                        mybir.ActivationFunctionType.Copy,
                        scale=cb_sb[:, b : b + 1],
                    )
                    t2 = wpool.tile([P, dout], dtype=F32, tag="t2")
                    nc.scalar.activation(
                        t2[:],
                        o2[:],
                        mybir.ActivationFunctionType.Copy,
                        scale=dv_sb[:, b : b + 1],
                    )
                    nc.vector.tensor_tensor(
                        out=osb_sb[:, j, :],
                        in0=t2[:],
                        in1=t1[:],
                        op=mybir.AluOpType.add,
                    )
                nc.sync.dma_start(
                    out=out_v[:, blks[0] : blks[0] + nb, :], in_=osb_sb[:, 0:nb, :]
                )
    nc.compile()
    return nc


def _prep(x, edge_index, W, b):
    N, din = x.shape
    dout = W.shape[0]
    npc = N // NCORES
    nblk = (npc + P - 1) // P
    npc_pad = nblk * P
    nchunk = (N + CHUNK - 1) // CHUNK
    n_y = nchunk * CHUNK

    row = np.asarray(edge_index[0], dtype=np.int64)
    col = np.asarray(edge_index[1], dtype=np.int64)
    deg = np.bincount(row, minlength=N)  # int
    d64 = 1.0 / np.sqrt(deg.astype(np.float64))
    d32 = d64.astype(np.float32)
    # cc[r] = sum_{edges (r,c)} d_c + d_r   (f64 accumulate on host)
    cc = np.bincount(row, weights=d64[col], minlength=N) + d64
    cbv = (cc * d64).astype(np.float32)  # coefficient of b per node

    order_e = np.argsort(row, kind="stable")
    row_s = row[order_e]
    col_s = col[order_e]
    rowstart = np.zeros(N + 1, dtype=np.int64)
    np.cumsum(deg, out=rowstart[1:])

    sblocks = [list(range(s, min(s + SB, nblk))) for s in range(0, nblk, SB)]
    nsb = len(sblocks)

    # ---- per-core edge lists (dest-sharded) --------------------------------
    # per core arrays sorted by (superblock, chunk, dest); dl = dest-local
    # within superblock (0..SB*128-1), cl = global source id.
    core_dl = []
    core_cl = []
    counts = np.zeros((NCORES, nsb, nchunk), dtype=np.int64)
    for m in range(NCORES):
        lo, hi = rowstart[m * npc], rowstart[(m + 1) * npc]
        dl = row_s[lo:hi] - m * npc
        cl = col_s[lo:hi]
        sbix = (dl >> 7) // SB
        ch = cl // CHUNK
        o = np.lexsort((dl, ch, sbix))
        dl, cl, sbix, ch = dl[o], cl[o], sbix[o], ch[o]
        core_dl.append(dl)
        core_cl.append(cl)
        np.add.at(counts[m], (sbix, ch), 1)

    ntl = (np.max(counts, axis=0) + P - 1) // P  # [nsb, nchunk] tiles
    # ---- static tile schedule ----------------------------------------------
    sb_calls = []
    sb_tiles = []
    tile_base = []
    tstart_slot = np.zeros(nsb * nchunk, dtype=np.int64)  # slot base per group
    gt = 0
    for sbi in range(nsb):
        tile_base.append(gt)
        calls = []
        toff = 0
        for c in range(nchunk):
            nt = int(ntl[sbi, c])
            if nt:
                calls.append((c, toff, nt))
                tstart_slot[sbi * nchunk + c] = (gt + toff) * P
                toff += nt
        sb_calls.append(calls)
        sb_tiles.append(toff)
        gt += toff
    ttot = gt

    # ---- per-core slot data -------------------------------------------------
    idx_all = np.zeros((NCORES, P, ttot * 8), dtype=np.int16)
    dlf_all = np.full((NCORES, ttot, P), -1.0, dtype=np.float32)
    dcf_all = np.zeros((NCORES, ttot, P), dtype=np.float32)
    for m in range(NCORES):
        dl, cl = core_dl[m], core_cl[m]
        sbix = (dl >> 7) // SB
        ch = cl // CHUNK
        gkey = sbix * nchunk + ch
        gcnt = np.bincount(gkey, minlength=nsb * nchunk)
        grp_start = np.zeros(nsb * nchunk + 1, dtype=np.int64)
        np.cumsum(gcnt, out=grp_start[1:])
        within = np.arange(len(dl), dtype=np.int64) - grp_start[gkey]
        slot = tstart_slot[gkey] + within
        tno = slot >> 7
        pno = slot & 127
        lcol = (cl - ch * CHUNK).astype(np.int16)
        # wrapped idx layout: value for slot j of tile t lives at
        # [16 rows](j%16), col t*8 + j//16, replicated over 8 groups of 16
        flat = np.zeros((ttot, P), dtype=np.int16)
        flat[tno, pno] = lcol
        wrapped = flat.reshape(ttot, 8, 16).transpose(2, 0, 1).reshape(16, ttot * 8)
        idx_all[m] = np.tile(wrapped, (8, 1))
        dlf_all[m][tno, pno] = (dl - sbix * (SB * P)).astype(np.float32)
        dcf_all[m][tno, pno] = d32[cl]

    # ---- per-tile, per-block union segments --------------------------------
    valid = dlf_all >= 0
    tile_segs = [[] for _ in range(ttot)]
    for j in range(SB):
        inblk = valid & (dlf_all >= j * P) & (dlf_all < (j + 1) * P)
        vmin = np.where(inblk, dlf_all, 1e9).min(axis=(0, 2))
        vmax = np.where(inblk, dlf_all, -1e9).max(axis=(0, 2))
        has = inblk.any(axis=(0, 2))
        for t in np.nonzero(has)[0]:
            lo = int(vmin[t]) - j * P
            w = int(vmax[t]) - int(vmin[t]) + 1
            tile_segs[int(t)].append((j, lo, w))
    for t in range(ttot):
        tile_segs[t].sort()

    dldc_all = np.empty((NCORES, P, 2 * ttot), dtype=np.float16)
    for m in range(NCORES):
        dldc_all[m, :, 0:ttot] = dlf_all[m].T
        dldc_all[m, :, ttot : 2 * ttot] = dcf_all[m].T

    # ---- per-node inputs ----------------------------------------------------
    xf = np.asarray(x, dtype=np.float32)
    x_full = np.zeros((n_y, din), dtype=np.float16)
    x_full[:N] = xf
    xs_all = np.zeros((NCORES, npc_pad, din), dtype=np.float16)
    dv_all = np.ones((NCORES, P, nblk), dtype=np.float32)
    cb_all = np.zeros((NCORES, P, nblk), dtype=np.float32)
    for m in range(NCORES):
        xs_all[m, :npc] = xf[m * npc : (m + 1) * npc]
        dm = np.ones(npc_pad, dtype=np.float32)
        dm[:npc] = d32[m * npc : (m + 1) * npc]
        dv_all[m] = dm.reshape(nblk, P).T
        cm_ = np.zeros(npc_pad, dtype=np.float32)
        cm_[:npc] = cbv[m * npc : (m + 1) * npc]
        cb_all[m] = cm_.reshape(nblk, P).T

    meta = dict(
        N=N, din=din, dout=dout, npc=npc, nblk=nblk, npc_pad=npc_pad,
        nchunk=nchunk, n_y=n_y, ttot=ttot,
        sblocks=sblocks, sb_tiles=sb_tiles, sb_calls=sb_calls,
        tile_base=tile_base, tile_segs=tile_segs,
    )
    data = dict(
        idx_all=idx_all, dldc_all=dldc_all,
        x_full=x_full, xs_all=xs_all, dv_all=dv_all, cb_all=cb_all,
    )
    return meta, data


def kernel(x, edge_index, W, b):
    x = np.asarray(x, dtype=np.float32)
    W = np.asarray(W, dtype=np.float32)
    b = np.asarray(b, dtype=np.float32)
    edge_index = np.asarray(edge_index)
    meta, data = _prep(x, edge_index, W, b)
    N, din, dout = meta["N"], meta["din"], meta["dout"]

    key = (
        "l", N, din, dout,
        tuple(int(t) for t in np.asarray(meta["sb_tiles"])),
        meta["ttot"],
        tuple(tuple(s) for t in meta["tile_segs"] for s in t),
    )
    if key not in _cache:
        _cache[key] = _build(meta)
    nc = _cache[key]

    wt = np.ascontiguousarray(W.T)
    brep = np.repeat(b[None, :], P, axis=0).astype(np.float32)
    in_maps = [
        {
            "x_t": data["x_full"],
            "xs_t": data["xs_all"][m],
            "idx_t": data["idx_all"][m],
            "dldc_t": data["dldc_all"][m],
            "dv_t": data["dv_all"][m],
            "cb_t": data["cb_all"][m],
            "wt_t": wt,
            "brep_t": brep,
        }
        for m in range(NCORES)
    ]
    res = run_bass_kernel_spmd(nc, in_maps, list(range(NCORES))).results

    LAST.clear()
    LAST.update(launches=[("launch", nc, in_maps)])

    out = np.empty((N, dout), dtype=np.float32)
    for m in range(NCORES):
        out[m * meta["npc"] : (m + 1) * meta["npc"]] = res[m]["out_t"][
            : meta["npc"]
        ].astype(np.float32)
    return out


# revision 34
# speedup vs baseline: 1.0181x; 1.0181x over previous
"""GCN layer kernel for Trainium2, 8 NeuronCores — single-launch version.

Math (identical to reference):
    deg = bincount(row);  d = 1/sqrt(deg)
    h   = x @ W.T + b
    out = d * segment_sum(d[col] * h[col], row) + d^2 * h

Rewritten as aggregate-then-transform (linear map commutes with segment sum):
    U[r]   = sum_{edges (r,c)} d_c * x_c + d_r * x_r
    cc[r]  = sum_{edges (r,c)} d_c + d_r
    out[r] = d_r * (U[r] @ W.T) + (cc[r] * d_r) * b

One SPMD launch (destinations sharded across the 8 cores, identical program,
per-core data):
  * edges sorted by (dest superblock of 4x128, source chunk of 25088, dest);
    x rows (fp16, 256B) gathered in bulk with gpsimd.dma_gather.  Gathered
    edge i lands at SBUF partition i%128, tile i//128.  Tiles are packed per
    (superblock, chunk) — NOT aligned to 128-dest blocks — so cross-core
    slot padding is only ~6%.
  * per 128-edge tile and dest block it touches, a selection matrix
    st[e, dest] = (iota==dl)*d_c (fp16, dl = dest-local within superblock,
    0..511) is built with one fused tensor_scalar (is_equal, mult) — carrying
    the per-source d_c scaling — and one PE fp16 matmul accumulates
    slab^T @ st into that block's f32 PSUM tile U^T[feat, dest].  The four
    blocks of a superblock accumulate concurrently in separate PSUM banks.
  * self term: one matmul per block with rhs = ident * d_r (diagonal of d)
    which also clears the full PSUM tile.
  * per block: U^T is already [feat, dest], so no transpose: one 128x128
    matmul with W^T, then scale by d_r and add (cc*d_r)*b (cc precomputed on
    host along with all index/degree prep).  Output written fp16, host
    upcasts.
Slot padding uses source row 0 with dl = -1 and d_c = 0 (selection column is
all zero), so padded gathers are harmless; per-(superblock,chunk) tile counts
are the max over cores, keeping shapes static across the SPMD program.
"""

import numpy as np
import sys

sys.path.insert(0, "/opt/trn_rl_repo")

import concourse.bacc as bacc
import concourse.tile as tile
from concourse import mybir
from concourse.bass_utils import run_bass_kernel_spmd
from concourse.masks import make_identity

NCORES = 8
P = 128
CHUNK = 25088  # dma_gather idx is int16: source chunks must stay < 32768 rows
SB = 4  # dest blocks per superblock (gather + tile-packing granularity)
BG = 4  # blocks processed per PSUM group (SB/BG sequential groups per sb)
SLAB_BUFS = 2
F32 = mybir.dt.float32
F16 = mybir.dt.float16
I16 = mybir.dt.int16

_cache = {}
LAST = {}  # populated on each kernel() call (for profiling in test.py)


def _build(meta):
    """Gather + selection-matmul segment sum + per-block W matmul."""
    din = meta["din"]
    dout = meta["dout"]
    n_y = meta["n_y"]  # padded x rows (nchunk * CHUNK)
    nblk = meta["nblk"]
    sblocks = meta["sblocks"]  # list of lists of block ids
    sb_tiles = meta["sb_tiles"]  # per sb: total tiles
    sb_calls = meta["sb_calls"]  # per sb: list of (chunk, tile_off_in_sb, ntiles)
    tile_base = meta["tile_base"]  # per sb: global tile offset
    tile_segs = meta["tile_segs"]  # per global tile: list of (j, lo, w)
    ttot = meta["ttot"]

    nc = bacc.Bacc(
        "TRN2",
        target_bir_lowering=False,
        debug=False,
        enable_asserts=False,
        num_devices=NCORES,
    )
    x_t = nc.dram_tensor("x_t", [n_y, din], F16, kind="ExternalInput").ap()
    xs_t = nc.dram_tensor("xs_t", [nblk * P, din], F16, kind="ExternalInput").ap()
    idx_t = nc.dram_tensor("idx_t", [P, ttot * 8], I16, kind="ExternalInput").ap()
    dldc_t = nc.dram_tensor("dldc_t", [P, 2 * ttot], F16, kind="ExternalInput").ap()
    dv_t = nc.dram_tensor("dv_t", [P, nblk], F32, kind="ExternalInput").ap()
    cb_t = nc.dram_tensor("cb_t", [P, nblk], F32, kind="ExternalInput").ap()
    wt_t = nc.dram_tensor("wt_t", [din, dout], F32, kind="ExternalInput").ap()
    brep_t = nc.dram_tensor("brep_t", [P, dout], F32, kind="ExternalInput").ap()
    out_t = nc.dram_tensor("out_t", [nblk * P, dout], F16, kind="ExternalOutput").ap()

    max_sb_tiles = max(sb_tiles)

    with tile.TileContext(nc) as tc:
        with (
            tc.tile_pool(name="const", bufs=1) as cpool,
            tc.tile_pool(name="slab", bufs=SLAB_BUFS) as gpool,
            tc.tile_pool(name="sel", bufs=6) as selpool,
            tc.tile_pool(name="work", bufs=3) as wpool,
            tc.tile_pool(name="psum_u", bufs=1, space="PSUM") as upool,
            tc.tile_pool(name="psum_o", bufs=2, space="PSUM") as opool,
        ):
            ident = cpool.tile([P, P], dtype=F16)
            make_identity(nc, ident[:])
            iota_i = cpool.tile([P, SB * P], dtype=mybir.dt.int32)
            nc.gpsimd.iota(
                iota_i[:], pattern=[[1, SB * P]], base=0, channel_multiplier=0
            )
            iota_f = cpool.tile([P, SB * P], dtype=F16)
            nc.vector.tensor_copy(iota_f[:], iota_i[:])
            wt_sb = cpool.tile([din, dout], dtype=F32)
            nc.sync.dma_start(out=wt_sb[:], in_=wt_t[:, :])
            brep_sb = cpool.tile([P, dout], dtype=F32)
            nc.sync.dma_start(out=brep_sb[:], in_=brep_t[:, :])
            dv_sb = cpool.tile([P, nblk], dtype=F32)
            nc.sync.dma_start(out=dv_sb[:], in_=dv_t[:, :])
            cb_sb = cpool.tile([P, nblk], dtype=F32)
            nc.sync.dma_start(out=cb_sb[:], in_=cb_t[:, :])
            # one-shot bulk loads: gather indices, dl/dc (f16 -> f32), self rows
            idx_all = cpool.tile([P, ttot * 8], dtype=I16)
            nc.sync.dma_start(out=idx_all[:], in_=idx_t[:, :])
            dldc_stg = cpool.tile([P, 2 * ttot], dtype=F16)
            nc.sync.dma_start(out=dldc_stg[:], in_=dldc_t[:, :])
            dl_f = cpool.tile([P, ttot], dtype=F32)
            nc.vector.tensor_copy(dl_f[:], dldc_stg[:, 0:ttot])
            dc_f = cpool.tile([P, ttot], dtype=F32)
            nc.vector.tensor_copy(dc_f[:], dldc_stg[:, ttot : 2 * ttot])

            xs_v = xs_t.rearrange("(t p) f -> p t f", p=P)
            xs_all = cpool.tile([P, nblk, din], dtype=F16)
            nc.sync.dma_start(out=xs_all[:], in_=xs_v[:, :, :])
            out_v = out_t.rearrange("(t p) f -> p t f", p=P)
            for sbi, blks in enumerate(sblocks):
                nt_sb = sb_tiles[sbi]
                tb = tile_base[sbi]
                nb = len(blks)
                # last edge-tile touching each block (for stop flags)
                last_t = [None] * nb
                for t_sb in range(nt_sb):
                    for (j, lo, w) in tile_segs[tb + t_sb]:
                        last_t[j] = t_sb
                slab = gpool.tile([P, max_sb_tiles, din], dtype=F16, tag="slab")
                for (c, toff, nt) in sb_calls[sbi]:
                    ni = nt * P
                    nc.gpsimd.dma_gather(
                        out_ap=slab[:, toff : toff + nt, :],
                        in_ap=x_t[c * CHUNK : (c + 1) * CHUNK, :],
                        idxs_ap=idx_all[:, (tb + toff) * 8 : (tb + toff + nt) * 8],
                        num_idxs=ni,
                        num_idxs_reg=ni,
                        elem_size=din,
                        single_packet=False,
                    )
                osb_sb = wpool.tile([P, SB, dout], dtype=F16, tag="osb")
                # blocks processed in groups of BG (each group: 4 PSUM banks)
                for g0 in range(0, nb, BG):
                    grp = list(range(g0, min(g0 + BG, nb)))
                    # PSUM tiles hold U^T: [feat, dest_local], one bank/block
                    ups = {}
                    for j in grp:
                        b = blks[j]
                        u = upool.tile(
                            [P, P], dtype=F32, space="PSUM", tag=f"ups{j - g0}"
                        )
                        ups[j] = u
                        # self term first: rhs = diag(d_r); clears the tile
                        dd = selpool.tile([P, P], dtype=F16, tag="dd")
                        nc.vector.tensor_scalar(
                            out=dd[:],
                            in0=ident[:],
                            scalar1=dv_sb[:, b : b + 1],
                            scalar2=None,
                            op0=mybir.AluOpType.mult,
                        )
                        nc.tensor.matmul(
                            out=u[:],
                            lhsT=xs_all[:, b, :],
                            rhs=dd[:],
                            start=True,
                            stop=(last_t[j] is None),
                        )
                    for t_sb in range(nt_sb):
                        for (j, lo, w) in tile_segs[tb + t_sb]:
                            if j not in ups:
                                continue
                            st = selpool.tile([P, P], dtype=F16, tag="st")
                            nc.vector.tensor_scalar(
                                out=st[:, 0:w],
                                in0=iota_f[:, j * P + lo : j * P + lo + w],
                                scalar1=dl_f[:, tb + t_sb : tb + t_sb + 1],
                                scalar2=dc_f[:, tb + t_sb : tb + t_sb + 1],
                                op0=mybir.AluOpType.is_equal,
                                op1=mybir.AluOpType.mult,
                            )
                            nc.tensor.matmul(
                                out=ups[j][:, lo : lo + w],
                                lhsT=slab[:, t_sb, :],
                                rhs=st[:, 0:w],
                                start=False,
                                stop=(t_sb == last_t[j]),
                            )
                    for j in grp:
                        b = blks[j]
                        # U^T -> SBUF, out = d_r * (U @ W^T) + (cc*d_r) * b
                        usb = wpool.tile([P, P], dtype=F32, tag="usb")
                        nc.scalar.activation(
                            usb[:], ups[j][:], mybir.ActivationFunctionType.Copy
                        )
                        o2 = opool.tile([P, dout], dtype=F32, space="PSUM", tag="o2")
                        nc.tensor.matmul(
                            out=o2[:], lhsT=usb[:], rhs=wt_sb[:], start=True, stop=True
                        )
                        t1 = wpool.tile([P, dout], dtype=F32, tag="t1")
                        nc.scalar.activation(
                            t1[:],
                            brep_sb[:],
                            mybir.ActivationFunctionType.Copy,
                            scale=cb_sb[:, b : b + 1],
                        )
                        t2 = wpool.tile([P, dout], dtype=F32, tag="t2")
                        nc.scalar.activation(
                            t2[:],
                            o2[:],
                            mybir.ActivationFunctionType.Copy,
                            scale=dv_sb[:, b : b + 1],
                        )
                        nc.vector.tensor_tensor(
                            out=osb_sb[:, j, :],
                            in0=t2[:],
                            in1=t1[:],
                            op=mybir.AluOpType.add,
                        )
                nc.sync.dma_start(
                    out=out_v[:, blks[0] : blks[0] + nb, :], in_=osb_sb[:, 0:nb, :]
                )
    nc.compile()
    return nc


def _prep(x, edge_index, W, b):
    N, din = x.shape
    dout = W.shape[0]
    npc = N // NCORES
    nblk = (npc + P - 1) // P
    npc_pad = nblk * P
    nchunk = (N + CHUNK - 1) // CHUNK
    n_y = nchunk * CHUNK

    row = np.asarray(edge_index[0], dtype=np.int64)
    col = np.asarray(edge_index[1], dtype=np.int64)
    deg = np.bincount(row, minlength=N)  # int
    d64 = 1.0 / np.sqrt(deg.astype(np.float64))
    d32 = d64.astype(np.float32)
    # cc[r] = sum_{edges (r,c)} d_c + d_r   (f64 accumulate on host)
    cc = np.bincount(row, weights=d64[col], minlength=N) + d64
    cbv = (cc * d64).astype(np.float32)  # coefficient of b per node

    order_e = np.argsort(row, kind="stable")
    row_s = row[order_e]
    col_s = col[order_e]
    rowstart = np.zeros(N + 1, dtype=np.int64)
    np.cumsum(deg, out=rowstart[1:])

    sblocks = [list(range(s, min(s + SB, nblk))) for s in range(0, nblk, SB)]
    nsb = len(sblocks)

    # ---- per-core edge lists (dest-sharded) --------------------------------
    # per core arrays sorted by (superblock, chunk, dest); dl = dest-local
    # within superblock (0..SB*128-1), cl = global source id.
    core_dl = []
    core_cl = []
    counts = np.zeros((NCORES, nsb, nchunk), dtype=np.int64)
    for m in range(NCORES):
        lo, hi = rowstart[m * npc], rowstart[(m + 1) * npc]
        dl = row_s[lo:hi] - m * npc
        cl = col_s[lo:hi]
        sbix = (dl >> 7) // SB
        ch = cl // CHUNK
        o = np.lexsort((dl, ch, sbix))
        dl, cl, sbix, ch = dl[o], cl[o], sbix[o], ch[o]
        core_dl.append(dl)
        core_cl.append(cl)
        np.add.at(counts[m], (sbix, ch), 1)

    ntl = (np.max(counts, axis=0) + P - 1) // P  # [nsb, nchunk] tiles
    # ---- static tile schedule ----------------------------------------------
    sb_calls = []
    sb_tiles = []
    tile_base = []
    tstart_slot = np.zeros(nsb * nchunk, dtype=np.int64)  # slot base per group
    gt = 0
    for sbi in range(nsb):
        tile_base.append(gt)
        calls = []
        toff = 0
        for c in range(nchunk):
            nt = int(ntl[sbi, c])
            if nt:
                calls.append((c, toff, nt))
                tstart_slot[sbi * nchunk + c] = (gt + toff) * P
                toff += nt
        sb_calls.append(calls)
        sb_tiles.append(toff)
        gt += toff
    ttot = gt

    # ---- per-core slot data -------------------------------------------------
    idx_all = np.zeros((NCORES, P, ttot * 8), dtype=np.int16)
    dlf_all = np.full((NCORES, ttot, P), -1.0, dtype=np.float32)
    dcf_all = np.zeros((NCORES, ttot, P), dtype=np.float32)
    for m in range(NCORES):
        dl, cl = core_dl[m], core_cl[m]
        sbix = (dl >> 7) // SB
        ch = cl // CHUNK
        gkey = sbix * nchunk + ch
        gcnt = np.bincount(gkey, minlength=nsb * nchunk)
        grp_start = np.zeros(nsb * nchunk + 1, dtype=np.int64)
        np.cumsum(gcnt, out=grp_start[1:])
        within = np.arange(len(dl), dtype=np.int64) - grp_start[gkey]
        slot = tstart_slot[gkey] + within
        tno = slot >> 7
        pno = slot & 127
        lcol = (cl - ch * CHUNK).astype(np.int16)
        # wrapped idx layout: value for slot j of tile t lives at
        # [16 rows](j%16), col t*8 + j//16, replicated over 8 groups of 16
        flat = np.zeros((ttot, P), dtype=np.int16)
        flat[tno, pno] = lcol
        wrapped = flat.reshape(ttot, 8, 16).transpose(2, 0, 1).reshape(16, ttot * 8)
        idx_all[m] = np.tile(wrapped, (8, 1))
        dlf_all[m][tno, pno] = (dl - sbix * (SB * P)).astype(np.float32)
        dcf_all[m][tno, pno] = d32[cl]

    # ---- per-tile, per-block union segments --------------------------------
    valid = dlf_all >= 0
    tile_segs = [[] for _ in range(ttot)]
    for j in range(SB):
        inblk = valid & (dlf_all >= j * P) & (dlf_all < (j + 1) * P)
        vmin = np.where(inblk, dlf_all, 1e9).min(axis=(0, 2))
        vmax = np.where(inblk, dlf_all, -1e9).max(axis=(0, 2))
        has = inblk.any(axis=(0, 2))
        for t in np.nonzero(has)[0]:
            lo = int(vmin[t]) - j * P
            w = int(vmax[t]) - int(vmin[t]) + 1
            tile_segs[int(t)].append((j, lo, w))
    for t in range(ttot):
        tile_segs[t].sort()

    dldc_all = np.empty((NCORES, P, 2 * ttot), dtype=np.float16)
    for m in range(NCORES):
        dldc_all[m, :, 0:ttot] = dlf_all[m].T
        dldc_all[m, :, ttot : 2 * ttot] = dcf_all[m].T

    # ---- per-node inputs ----------------------------------------------------
    xf = np.asarray(x, dtype=np.float32)
    x_full = np.zeros((n_y, din), dtype=np.float16)
    x_full[:N] = xf
    xs_all = np.zeros((NCORES, npc_pad, din), dtype=np.float16)
    dv_all = np.ones((NCORES, P, nblk), dtype=np.float32)
    cb_all = np.zeros((NCORES, P, nblk), dtype=np.float32)
    for m in range(NCORES):
        xs_all[m, :npc] = xf[m * npc : (m + 1) * npc]
        dm = np.ones(npc_pad, dtype=np.float32)
        dm[:npc] = d32[m * npc : (m + 1) * npc]
        dv_all[m] = dm.reshape(nblk, P).T
        cm_ = np.zeros(npc_pad, dtype=np.float32)
        cm_[:npc] = cbv[m * npc : (m + 1) * npc]
        cb_all[m] = cm_.reshape(nblk, P).T

    meta = dict(
        N=N, din=din, dout=dout, npc=npc, nblk=nblk, npc_pad=npc_pad,
        nchunk=nchunk, n_y=n_y, ttot=ttot,
        sblocks=sblocks, sb_tiles=sb_tiles, sb_calls=sb_calls,
        tile_base=tile_base, tile_segs=tile_segs,
    )
    data = dict(
        idx_all=idx_all, dldc_all=dldc_all,
        x_full=x_full, xs_all=xs_all, dv_all=dv_all, cb_all=cb_all,
    )
    return meta, data


def kernel(x, edge_index, W, b):
    x = np.asarray(x, dtype=np.float32)
    W = np.asarray(W, dtype=np.float32)
    b = np.asarray(b, dtype=np.float32)
    edge_index = np.asarray(edge_index)
    meta, data = _prep(x, edge_index, W, b)
    N, din, dout = meta["N"], meta["din"], meta["dout"]

    key = (
        "l", N, din, dout,
        tuple(int(t) for t in np.asarray(meta["sb_tiles"])),
        meta["ttot"],
        tuple(tuple(s) for t in meta["tile_segs"] for s in t),
    )
    if key not in _cache:
        _cache[key] = _build(meta)
    nc = _cache[key]

    wt = np.ascontiguousarray(W.T)
    brep = np.repeat(b[None, :], P, axis=0).astype(np.float32)
    in_maps = [
        {
            "x_t": data["x_full"],
            "xs_t": data["xs_all"][m],
            "idx_t": data["idx_all"][m],
            "dldc_t": data["dldc_all"][m],
            "dv_t": data["dv_all"][m],
            "cb_t": data["cb_all"][m],
            "wt_t": wt,
            "brep_t": brep,
        }
        for m in range(NCORES)
    ]
    res = run_bass_kernel_spmd(nc, in_maps, list(range(NCORES))).results

    LAST.clear()
    LAST.update(launches=[("launch", nc, in_maps)])

    out = np.empty((N, dout), dtype=np.float32)
    for m in range(NCORES):
        out[m * meta["npc"] : (m + 1) * meta["npc"]] = res[m]["out_t"][
            : meta["npc"]
        ].astype(np.float32)
    return out
